# revision 13
# baseline (speedup 1.0000x reference)
"""Trainium2 Bass kernel for nn_Attention (dense transformer attention layer).

Reference semantics (bug-faithful to the source):
  - Q = x @ wq.T ; V = x @ wv.T ; K-projection is DEAD CODE (the reference
    overwrites xk with the double-angle-rotated Q, so wk never matters).
  - rot = double-angle RoPE applied to Q; keys == rot(Q).
  - start_pos == 0 and t == MAX_SEQ, so the KV cache contents never matter.
  - scores = rotQ @ rotQ.T / sqrt(HD) + mask ; P = softmax ; O = P @ V
  - out = O @ wo.T

Sharding (8 cores): core c -> batch b = c//2, head-half h = c%2 (8 of 16
heads).  Q/V projections + attention are (batch x head-half) parallel; each
core AllGathers its full-T per-head attention output within its pair, then
runs the output projection on its token-half with a full-D contraction.

Schedule (single TileContext, dataflow-overlapped):
  1. V projection for all tokens (x-stationary matmuls), releases wv.
  2. Per token-quarter n: Q^T projection (weight-stationary matmuls produce
     Q^T [feat, tok] directly -- no PE transposes; RoPE runs in transposed
     layout on r|i partition halves, enabled by a host-side deinterleaving
     column permutation of wq) followed by attention chunk n of all heads.
  3. Attention: scores in [k, q] layout (symmetric Gram matrix), exp on the
     Scalar engine over 2-bank PSUM mega-tiles, PV accumulation in PSUM.
     Softmax denominators never touch the PE: exp tiles are accumulated on
     the Vector engine and partition-reduced on GpSimd (partition_all_reduce).
  4. Per-head AllGather (pairs) of full-T rows, then the output projection.

All large inputs are host-cast to bf16 (compute dtype) to halve HBM traffic.
"""

import math
import sys

import numpy as np

sys.path.insert(0, "/opt/trn_rl_repo")

import concourse.bacc as bacc
import concourse.mybir as mybir
from concourse import bass_isa
from concourse.tile import TileContext

F32 = mybir.dt.float32
BF16 = mybir.dt.bfloat16

B = 4
T = 2048
D = 2048
H = 16
HD = 128
N_CORES = 8
PAIRS = [[0, 1], [2, 3], [4, 5], [6, 7]]


def build_nc(T, D, H):
    HD = 128
    assert D == H * HD
    NH = H // 2          # heads per core (8)
    DQ = NH * HD         # own q/v feature count (1024)
    TH = T // 2          # token half
    NT = T // 128        # token tiles (16)
    ND = D // 128        # d tiles (16)
    NQ = 4               # token quarters
    XTQ = T // NQ        # tokens per quarter (512)
    QPC = XTQ // 128     # token tiles per quarter (4)
    CH = XTQ             # attention q-chunk width (512)
    scale = 1.0 / math.sqrt(HD)

    nc = bacc.Bacc(target_bir_lowering=False, num_devices=N_CORES)

    xt = nc.declare_dram_parameter("xt", [D, T], BF16, isOutput=False)
    wqt = nc.declare_dram_parameter("wqt", [D, DQ], BF16, isOutput=False)
    wvt = nc.declare_dram_parameter("wvt", [D, DQ], BF16, isOutput=False)
    wot = nc.declare_dram_parameter("wot", [D, D], BF16, isOutput=False)
    mkt = nc.declare_dram_parameter("maskt", [128, 128], F32, isOutput=False)
    fcs = nc.declare_dram_parameter("fcs", [64, 2 * T], F32, isOutput=False)
    out = nc.declare_dram_parameter("out", [TH, D], F32, isOutput=True)

    # per-head pair exchange of full-T attention output rows
    ag_in = [nc.dram_tensor(f"agi{e}", [128, T], BF16) for e in range(NH)]
    ag_out = [nc.dram_tensor(f"ago{e}", [2, 128, T], BF16) for e in range(NH)]

    with TileContext(nc) as tc:
        import concourse.bass as bass_mod

        pid = nc.partition_id()
        h_idx = pid % 2
        off_own = h_idx * TH         # this core's token-half offset
        peer_i = 1 - h_idx           # peer's index within the pair

        # ---------------- persistent / long-lived pools -----------------
        _cm = {}

        def popen(name, **kw):
            cm = tc.tile_pool(name=name, **kw)
            _cm[name] = cm
            return cm.__enter__()

        def pclose(name):
            _cm.pop(name).__exit__(None, None, None)

        p_c2s2 = popen("c2s2", bufs=1)
        p_rotqt = popen("rotqt", bufs=1)
        p_vsb = popen("vsb", bufs=1)
        p_wqt = popen("wqt", bufs=1)
        p_xt = popen("xt", bufs=2)
        p_misc = popen("misc", bufs=1)
        # projection PSUM pool (used by both V and Q phases)
        p_proj = popen("projps", bufs=2, space="PSUM")
        p_wvt = popen("wvt", bufs=1)

        rotqt = p_rotqt.tile([128, NH * T], BF16, tag="rotqt")
        v_sb = p_vsb.tile([128, NT * DQ], BF16, tag="vsb")
        # double-angle tables, both on partitions 0:64 (cols 0:T c2, T:2T s2)
        c2s2 = p_c2s2.tile([64, 2 * T], F32, tag="c2s2")
        mkt_sb = p_misc.tile([128, 128], F32, tag="mkt")
        nc.sync.dma_start(out=mkt_sb[:, :], in_=mkt[:, :])

        # rope tables from packed fc|fs input (cols 0:T fc, T:2T fs)
        with tc.tile_pool(name="fcfs", bufs=1) as p_fcfs:
            fcfs = p_fcfs.tile([64, 2 * T], F32, tag="fcfs")
            nc.sync.dma_start(out=fcfs[:, :], in_=fcs[:, :])
            fc = fcfs[:, 0:T]
            fs = fcfs[:, T : 2 * T]
            c2 = c2s2[:, 0:T]
            s2 = c2s2[:, T : 2 * T]
            nc.vector.tensor_mul(c2, fc, fc)
            nc.vector.tensor_mul(s2, fs, fs)
            nc.vector.tensor_sub(c2, c2, s2)
            nc.vector.tensor_mul(s2, fc, fs)
            nc.vector.tensor_scalar_mul(s2, s2, 2.0)

        # weights: single 3D-AP DMAs
        wvt_sb = p_wvt.tile([128, ND * DQ], BF16, tag="wvt")
        nc.sync.dma_start(
            out=wvt_sb[:, :].rearrange("p (dk c) -> p dk c", dk=ND),
            in_=wvt[:, :].rearrange("(dk p) c -> p dk c", dk=ND),
        )
        wqt_sb = p_wqt.tile([128, ND * DQ], BF16, tag="wqt")
        nc.gpsimd.dma_start(
            out=wqt_sb[:, :].rearrange("p (dk c) -> p dk c", dk=ND),
            in_=wqt[:, :].rearrange("(dk p) c -> p dk c", dk=ND),
        )

        def load_xt_quarter(n):
            xt_sb = p_xt.tile([128, ND * XTQ], BF16, tag="xtq")
            nc.sync.dma_start(
                out=xt_sb[:, :].rearrange("p (dk t) -> p dk t", dk=ND),
                in_=xt[:, n * XTQ : (n + 1) * XTQ].rearrange(
                    "(dk p) t -> p dk t", dk=ND
                ),
            )
            return xt_sb

        # ---------------- phase 1: V projection (all tokens) -------------
        for n in range(NQ):
            xt_sb = load_xt_quarter(n)
            for j in range(QPC):
                tb = n * QPC + j
                for qc in range(2):
                    ps_v = p_proj.tile([128, 512], F32, tag="ps")
                    for dk in range(ND):
                        nc.tensor.matmul(
                            ps_v[:, :],
                            xt_sb[:, dk * XTQ + j * 128 : dk * XTQ + (j + 1) * 128],
                            wvt_sb[:, dk * DQ + qc * 512 : dk * DQ + (qc + 1) * 512],
                            start=(dk == 0),
                            stop=(dk == ND - 1),
                        )
                    nc.vector.tensor_copy(
                        v_sb[:, tb * DQ + qc * 512 : tb * DQ + (qc + 1) * 512],
                        ps_v[:, :],
                    )
        pclose("wvt")

        # attention-phase pools (SBUF ring space freed by wvt)
        p_tt = popen("ttmp", bufs=2)
        p_pt = popen("pt", bufs=3)
        p_acc = popen("acc", bufs=2)
        p_rcp = popen("rcp", bufs=2)
        p_otc = popen("otc", bufs=3)
        p_psS = popen("psS", bufs=2, space="PSUM")
        p_psO = popen("psO", bufs=2, space="PSUM")

        # ---------------- phase 2: Q^T + rope, interleaved attention ------
        def q_quarter(n, xt_sb):
            nsl = slice(n * XTQ, (n + 1) * XTQ)
            for f in range(NH):
                ps_q = p_proj.tile([128, 512], F32, tag="ps")
                for dk in range(ND):
                    nc.tensor.matmul(
                        ps_q[:, :],
                        wqt_sb[:, dk * DQ + f * 128 : dk * DQ + (f + 1) * 128],
                        xt_sb[:, dk * XTQ : (dk + 1) * XTQ],
                        start=(dk == 0),
                        stop=(dk == ND - 1),
                    )
                # rope in [feat, tok] layout: rows 0:64 real, 64:128 imag.
                # muls read PSUM+SBUF (mixed spaces, base-partition rule
                # exempt); the final sub/add reads two base-0 SBUF temps.
                qr = ps_q[0:64, :]
                qi = ps_q[64:128, :]
                c2n = c2s2[:, n * XTQ : (n + 1) * XTQ]
                s2n = c2s2[:, T + n * XTQ : T + (n + 1) * XTQ]
                col = slice(f * T + n * XTQ, f * T + (n + 1) * XTQ)
                t1a = p_tt.tile([64, 512], F32, tag="a")
                t1b = p_tt.tile([64, 512], F32, tag="b")
                nc.vector.tensor_mul(t1a[:, :], qr, c2n)
                nc.vector.tensor_mul(t1b[:, :], qi, s2n)
                nc.vector.tensor_sub(rotqt[0:64, col], t1a[:, :], t1b[:, :])
                t2a = p_tt.tile([64, 512], F32, tag="c")
                t2b = p_tt.tile([64, 512], F32, tag="d")
                nc.vector.tensor_mul(t2a[:, :], qr, s2n)
                nc.vector.tensor_mul(t2b[:, :], qi, c2n)
                nc.vector.tensor_add(rotqt[64:128, col], t2a[:, :], t2b[:, :])

        def attn_chunk(c):
            KC = (c + 1) * QPC
            q0 = c * CH
            for eta in range(NH):
                acc = p_acc.tile([128, CH], F32, tag="acc")
                ps_o = p_psO.tile([128, CH], F32, tag="pso")
                for m in range(KC // 2):
                    ps_s = p_psS.tile([128, 2 * CH], F32, tag="pss")
                    pt = p_pt.tile([128, 2 * CH], BF16, tag="pt")
                    for s in range(2):
                        kt = 2 * m + s
                        qo = max(0, (kt - c * QPC) * 128)
                        nc.tensor.matmul(
                            ps_s[:, s * CH + qo : (s + 1) * CH],
                            rotqt[:, eta * T + kt * 128 : eta * T + kt * 128 + 128],
                            rotqt[:, eta * T + q0 + qo : eta * T + q0 + CH],
                            start=True,
                            stop=True,
                        )
                        if kt >= c * QPC:  # diagonal block: apply causal mask
                            nc.vector.tensor_add(
                                ps_s[:, s * CH + qo : s * CH + qo + 128],
                                ps_s[:, s * CH + qo : s * CH + qo + 128],
                                mkt_sb[:, :],
                            )
                    # one exp over the whole 2-bank mega-tile (unwritten
                    # columns hold stale-but-finite data and are never read)
                    nc.scalar.activation(
                        pt[:, :],
                        ps_s[:, :],
                        mybir.ActivationFunctionType.Exp,
                        scale=scale,
                    )
                    for s in range(2):
                        kt = 2 * m + s
                        qo = max(0, (kt - c * QPC) * 128)
                        nc.tensor.matmul(
                            ps_o[:, qo:CH],
                            v_sb[:, kt * DQ + eta * 128 : kt * DQ + eta * 128 + 128],
                            pt[:, s * CH + qo : (s + 1) * CH],
                            start=(kt == 0),
                            stop=(kt == KC - 1),
                        )
                        if kt == 0:
                            nc.vector.tensor_copy(acc[:, :], pt[:, 0:CH])
                        else:
                            nc.vector.tensor_add(
                                acc[:, qo:CH],
                                acc[:, qo:CH],
                                pt[:, s * CH + qo : (s + 1) * CH],
                            )
                sums = p_rcp.tile([128, CH], F32, tag="sums")
                rcpb = p_rcp.tile([128, CH], F32, tag="rcpb")
                nc.gpsimd.partition_all_reduce(
                    sums[:, :], acc[:, :], channels=128,
                    reduce_op=bass_isa.ReduceOp.add,
                )
                nc.vector.reciprocal_approx_fast(rcpb[:, :], sums[:, :])
                otc = p_otc.tile([128, CH], BF16, tag="otc")
                nc.vector.tensor_mul(otc[:, :], ps_o[:, :], rcpb[:, :])
                nc.sync.dma_start(out=ag_in[eta][:, q0 : q0 + CH], in_=otc[:, :])
                if c == NQ - 1:
                    nc.gpsimd.collective_compute(
                        "AllGather",
                        mybir.AluOpType.bypass,
                        replica_groups=PAIRS,
                        ins=[ag_in[eta].ap().opt()],
                        outs=[ag_out[eta].ap().opt()],
                    )

        for n in range(NQ):
            xt_sb = load_xt_quarter(n)
            q_quarter(n, xt_sb)
            attn_chunk(n)

        # release all phase-1/2 pools (LIFO per space) before phase 3 opens
        for name in ("psO", "psS", "otc", "rcp", "acc", "pt", "ttmp",
                     "projps", "misc", "xt", "wqt", "vsb", "rotqt", "c2s2"):
            pclose(name)

        # ---------------- phase 3: gather + output projection -------------
        p_ofull = popen("ofull", bufs=1)
        p_wot = popen("wot", bufs=2)
        p_osb = popen("osb", bufs=3)
        p_psOut = popen("psOut", bufs=2, space="PSUM")

        o_full = p_ofull.tile([128, 2 * NH * TH], BF16, tag="ofull")
        for r16 in range(2 * NH):
            eta = r16 % NH
            src_i = h_idx if r16 < NH else peer_i
            nc.gpsimd.dma_start(
                out=o_full[:, r16 * TH : (r16 + 1) * TH],
                in_=ag_out[eta][
                    bass_mod.ds(src_i, 1), :, bass_mod.ds(off_own, TH)
                ],
            )

        NDO = D // 512
        for do in range(NDO):
            wot_sb = p_wot.tile([128, ND * 512], BF16, tag="wot")
            nc.sync.dma_start(
                out=wot_sb[:, :].rearrange("p (dk c) -> p dk c", dk=ND),
                in_=wot[:, do * 512 : (do + 1) * 512].rearrange(
                    "(dk p) c -> p dk c", dk=ND
                ),
            )
            for tb8 in range(TH // 128):
                ps_out = p_psOut.tile([128, 512], F32, tag="psout")
                for r16 in range(2 * NH):
                    nc.tensor.matmul(
                        ps_out[:, :],
                        o_full[:, r16 * TH + tb8 * 128 : r16 * TH + tb8 * 128 + 128],
                        wot_sb[:, r16 * 512 : (r16 + 1) * 512],
                        start=(r16 == 0),
                        stop=(r16 == 2 * NH - 1),
                    )
                osb = p_osb.tile([128, 512], F32, tag="osb")
                nc.vector.tensor_copy(osb[:, :], ps_out[:, :])
                nc.sync.dma_start(
                    out=out[tb8 * 128 : (tb8 + 1) * 128, do * 512 : (do + 1) * 512],
                    in_=osb[:, :],
                )

        for name in reversed(list(_cm)):
            pclose(name)

    nc.finalize()
    return nc


def host_prep(T, D, H, x, wq, wv, wo, mask, freqs_cos, freqs_sin):
    """Build per-core in_maps (host-side layout/dtype prep only)."""
    import ml_dtypes

    bf16 = ml_dtypes.bfloat16
    HD = 128
    NH = H // 2
    DQ = NH * HD
    mkt = np.ascontiguousarray(np.asarray(mask, np.float32).reshape(T, T)[:128, :128].T)
    fcn = np.asarray(freqs_cos, np.float32)  # [T, 64]
    fsn = np.asarray(freqs_sin, np.float32)
    fcs = np.ascontiguousarray(np.concatenate([fcn.T, fsn.T], axis=1))  # [64, 2T]
    # deinterleave permutation: within each head block, (r0,r1,..,i0,i1,..)
    perm = np.concatenate(
        [hb * 128 + np.r_[0:128:2, 1:128:2] for hb in range(NH)]
    )
    wot_full = np.asarray(wo, np.float32).T  # [din2, dout]
    in_maps = []
    for c in range(N_CORES):
        b, h = c // 2, c % 2
        rows = slice(h * DQ, (h + 1) * DQ)
        wqt_c = np.asarray(wq[rows], np.float32).T[:, perm]
        # o_full rows are in local head order (own heads first): permute wot
        wot_c = np.concatenate(
            [wot_full[h * DQ : (h + 1) * DQ], wot_full[(1 - h) * DQ : (2 - h) * DQ]],
            axis=0,
        )
        in_maps.append(
            {
                "xt": np.ascontiguousarray(np.asarray(x[b], np.float32).T.astype(bf16)),
                "wqt": np.ascontiguousarray(wqt_c.astype(bf16)),
                "wvt": np.ascontiguousarray(
                    np.asarray(wv[rows], np.float32).T.astype(bf16)
                ),
                "wot": np.ascontiguousarray(wot_c.astype(bf16)),
                "maskt": mkt,
                "fcs": fcs,
            }
        )
    return in_maps


_NC_CACHE = {}


def run(T, D, H, inputs, trace=False):
    from concourse.bass_utils import run_bass_kernel_spmd

    key = (T, D, H)
    if key not in _NC_CACHE:
        _NC_CACHE[key] = build_nc(T, D, H)
    nc = _NC_CACHE[key]
    in_maps = host_prep(
        T, D, H,
        inputs["x"], inputs["wq"], inputs["wv"], inputs["wo"],
        inputs["mask"], inputs["freqs_cos"], inputs["freqs_sin"],
    )
    res = run_bass_kernel_spmd(nc, in_maps, core_ids=list(range(N_CORES)), trace=trace)
    B_ = np.asarray(inputs["x"]).shape[0]
    TH = T // 2
    out = np.empty((B_, T, D), np.float32)
    for c in range(N_CORES):
        b, h = c // 2, c % 2
        out[b, h * TH : (h + 1) * TH, :] = res.results[c]["out"]
    return out, res


def kernel(**inputs):
    out, _ = run(T, D, H, inputs, trace=False)
    return out


# revision 22
# speedup vs baseline: 1.1038x; 1.1038x over previous
"""Trainium2 Bass kernel for nn_Attention (dense transformer attention layer).

Reference semantics (bug-faithful to the source):
  - Q = x @ wq.T ; V = x @ wv.T ; K-projection is DEAD CODE (the reference
    overwrites xk with the double-angle-rotated Q, so wk never matters).
  - rot = double-angle RoPE applied to Q; keys == rot(Q).
  - start_pos == 0 and t == MAX_SEQ, so the KV cache contents never matter.
  - scores = rotQ @ rotQ.T / sqrt(HD) + mask ; P = softmax ; O = P @ V
  - out = O @ wo.T

Sharding (8 cores): core c -> batch b = c//2, head-half h = c%2 (8 of 16
heads).  Q/V projections + attention are (batch x head-half) parallel; each
core AllGathers its full-T per-head attention output within its pair, then
runs the output projection on its token-half with a full-D contraction.

Schedule (single TileContext, dataflow-overlapped):
  1. V projection for all tokens (x-stationary matmuls), releases wv.
  2. Per token-quarter n: Q^T projection (weight-stationary matmuls produce
     Q^T [feat, tok] directly -- no PE transposes; RoPE runs in transposed
     layout on r|i partition halves, enabled by a host-side deinterleaving
     column permutation of wq) followed by attention chunk n of all heads.
  3. Attention: scores in [k, q] layout (symmetric Gram matrix), exp on the
     Scalar engine over 2-bank PSUM mega-tiles, PV accumulation in PSUM.
     Softmax denominators never touch the PE: exp tiles are accumulated on
     the Vector engine and partition-reduced on GpSimd (partition_all_reduce).
  4. Per-head AllGather (pairs) of full-T rows, then the output projection.

All large inputs are host-cast to bf16 (compute dtype) to halve HBM traffic.
"""

import math
import sys

import numpy as np

sys.path.insert(0, "/opt/trn_rl_repo")

import concourse.bacc as bacc
import concourse.mybir as mybir
from concourse import bass_isa
from concourse.tile import TileContext

F32 = mybir.dt.float32
BF16 = mybir.dt.bfloat16

B = 4
T = 2048
D = 2048
H = 16
HD = 128
N_CORES = 8
PAIRS = [[0, 1], [2, 3], [4, 5], [6, 7]]


def build_nc(T, D, H):
    HD = 128
    assert D == H * HD
    NH = H // 2          # heads per core (8)
    DQ = NH * HD         # own q/v feature count (1024)
    TH = T // 2          # token half
    NT = T // 128        # token tiles (16)
    ND = D // 128        # d tiles (16)
    NQ = 4               # token quarters
    XTQ = T // NQ        # tokens per quarter (512)
    QPC = XTQ // 128     # token tiles per quarter (4)
    CH = XTQ             # attention q-chunk width (512)
    scale = 1.0 / math.sqrt(HD)

    nc = bacc.Bacc(target_bir_lowering=False, num_devices=N_CORES)

    xt = nc.declare_dram_parameter("xt", [D, T], BF16, isOutput=False)
    wqt = nc.declare_dram_parameter("wqt", [D, DQ], BF16, isOutput=False)
    wvt = nc.declare_dram_parameter("wvt", [D, DQ], BF16, isOutput=False)
    wot = nc.declare_dram_parameter("wot", [D, D], BF16, isOutput=False)
    mkt = nc.declare_dram_parameter("maskt", [128, 128], F32, isOutput=False)
    fcs = nc.declare_dram_parameter("fcs", [64, 2 * T], F32, isOutput=False)
    out = nc.declare_dram_parameter("out", [TH, D], F32, isOutput=True)

    # per-(head, chunk) pair exchange of attention output rows (chunk-major
    # so each chunk's collective operates on a contiguous block)
    ag_in = [nc.dram_tensor(f"agi{e}", [4, 128, T // 4], BF16) for e in range(NH)]
    ag_out = [nc.dram_tensor(f"ago{e}", [4, 2, 128, T // 4], BF16) for e in range(NH)]

    with TileContext(nc) as tc:
        import concourse.bass as bass_mod

        pid = nc.partition_id()
        h_idx = pid % 2
        off_own = h_idx * TH         # this core's token-half offset
        peer_i = 1 - h_idx           # peer's index within the pair

        # ---------------- persistent / long-lived pools -----------------
        _cm = {}

        def popen(name, **kw):
            cm = tc.tile_pool(name=name, **kw)
            _cm[name] = cm
            return cm.__enter__()

        def pclose(name):
            _cm.pop(name).__exit__(None, None, None)

        p_c2s2 = popen("c2s2", bufs=1)
        p_rotqt = popen("rotqt", bufs=1)
        p_vsb = popen("vsb", bufs=1)
        p_wqt = popen("wqt", bufs=1)
        p_xt = popen("xt", bufs=2)
        p_misc = popen("misc", bufs=1)
        # projection PSUM pool (used by both V and Q phases)
        p_proj = popen("projps", bufs=2, space="PSUM")
        p_wvt = popen("wvt", bufs=1)

        rotqt = p_rotqt.tile([128, NH * T], BF16, tag="rotqt")
        v_sb = p_vsb.tile([128, NT * DQ], BF16, tag="vsb")
        # double-angle tables, both on partitions 0:64 (cols 0:T c2, T:2T s2)
        c2s2 = p_c2s2.tile([64, 2 * T], F32, tag="c2s2")
        mkt_sb = p_misc.tile([128, 128], F32, tag="mkt")
        ones_sb = p_misc.tile([128, 1], BF16, tag="ones")
        nc.sync.dma_start(out=mkt_sb[:, :], in_=mkt[:, :])
        nc.vector.memset(ones_sb[:, :], 1.0)

        # rope tables from packed fc|fs input (cols 0:T fc, T:2T fs)
        with tc.tile_pool(name="fcfs", bufs=1) as p_fcfs:
            fcfs = p_fcfs.tile([64, 2 * T], F32, tag="fcfs")
            nc.sync.dma_start(out=fcfs[:, :], in_=fcs[:, :])
            fc = fcfs[:, 0:T]
            fs = fcfs[:, T : 2 * T]
            c2 = c2s2[:, 0:T]
            s2 = c2s2[:, T : 2 * T]
            nc.vector.tensor_mul(c2, fc, fc)
            nc.vector.tensor_mul(s2, fs, fs)
            nc.vector.tensor_sub(c2, c2, s2)
            nc.vector.tensor_mul(s2, fc, fs)
            nc.vector.tensor_scalar_mul(s2, s2, 2.0)

        # weights: 3D-AP DMAs, halves spread across trigger queues so the
        # first V matmuls aren't gated on the full load
        HK = ND // 2
        wvt_sb = p_wvt.tile([128, ND * DQ], BF16, tag="wvt")
        wqt_sb = p_wqt.tile([128, ND * DQ], BF16, tag="wqt")
        for h2, q_eng in ((0, nc.sync), (1, nc.scalar)):
            q_eng.dma_start(
                out=wvt_sb[:, h2 * HK * DQ : (h2 + 1) * HK * DQ].rearrange(
                    "p (dk c) -> p dk c", dk=HK
                ),
                in_=wvt[h2 * HK * 128 : (h2 + 1) * HK * 128, :].rearrange(
                    "(dk p) c -> p dk c", dk=HK
                ),
            )
        for h2, q_eng in ((0, nc.scalar), (1, nc.gpsimd)):
            q_eng.dma_start(
                out=wqt_sb[:, h2 * HK * DQ : (h2 + 1) * HK * DQ].rearrange(
                    "p (dk c) -> p dk c", dk=HK
                ),
                in_=wqt[h2 * HK * 128 : (h2 + 1) * HK * 128, :].rearrange(
                    "(dk p) c -> p dk c", dk=HK
                ),
            )

        def load_xt_quarter(n, q_eng):
            xt_sb = p_xt.tile([128, ND * XTQ], BF16, tag="xtq")
            for h2 in (0, 1):
                q_eng.dma_start(
                    out=xt_sb[:, h2 * HK * XTQ : (h2 + 1) * HK * XTQ].rearrange(
                        "p (dk t) -> p dk t", dk=HK
                    ),
                    in_=xt[
                        h2 * HK * 128 : (h2 + 1) * HK * 128,
                        n * XTQ : (n + 1) * XTQ,
                    ].rearrange("(dk p) t -> p dk t", dk=HK),
                )
            return xt_sb

        # ---------------- phase 1: V projection (all tokens) -------------
        for n in range(NQ):
            xt_sb = load_xt_quarter(n, nc.gpsimd if n % 2 else nc.sync)
            for j in range(QPC):
                tb = n * QPC + j
                for qc in range(2):
                    ps_v = p_proj.tile([128, 512], F32, tag="ps")
                    for dk in range(ND):
                        nc.tensor.matmul(
                            ps_v[:, :],
                            xt_sb[:, dk * XTQ + j * 128 : dk * XTQ + (j + 1) * 128],
                            wvt_sb[:, dk * DQ + qc * 512 : dk * DQ + (qc + 1) * 512],
                            start=(dk == 0),
                            stop=(dk == ND - 1),
                        )
                    nc.vector.tensor_copy(
                        v_sb[:, tb * DQ + qc * 512 : tb * DQ + (qc + 1) * 512],
                        ps_v[:, :],
                    )
        pclose("wvt")

        # attention-phase pools (SBUF ring space freed by wvt)
        p_tt = popen("ttmp", bufs=2)
        p_pt = popen("pt", bufs=5)
        p_rcp = popen("rcp", bufs=2)
        p_otc = popen("otc", bufs=3)
        p_psS = popen("psS", bufs=3, space="PSUM")
        p_psO = popen("psO", bufs=2, space="PSUM")
        p_psD = popen("psD", bufs=1, space="PSUM")

        # ---------------- phase 2: Q^T + rope, interleaved attention ------
        def q_quarter(n, xt_sb):
            nsl = slice(n * XTQ, (n + 1) * XTQ)
            for f in range(NH):
                ps_q = p_proj.tile([128, 512], F32, tag="ps")
                for dk in range(ND):
                    nc.tensor.matmul(
                        ps_q[:, :],
                        wqt_sb[:, dk * DQ + f * 128 : dk * DQ + (f + 1) * 128],
                        xt_sb[:, dk * XTQ : (dk + 1) * XTQ],
                        start=(dk == 0),
                        stop=(dk == ND - 1),
                    )
                # rope in [feat, tok] layout: rows 0:64 real, 64:128 imag.
                # muls read PSUM+SBUF (mixed spaces, base-partition rule
                # exempt); the final sub/add reads two base-0 SBUF temps.
                qr = ps_q[0:64, :]
                qi = ps_q[64:128, :]
                c2n = c2s2[:, n * XTQ : (n + 1) * XTQ]
                s2n = c2s2[:, T + n * XTQ : T + (n + 1) * XTQ]
                col = slice(f * T + n * XTQ, f * T + (n + 1) * XTQ)
                t1a = p_tt.tile([64, 512], BF16, tag="a")
                t1b = p_tt.tile([64, 512], BF16, tag="b")
                nc.vector.tensor_mul(t1a[:, :], qr, c2n)
                nc.vector.tensor_mul(t1b[:, :], qi, s2n)
                nc.vector.tensor_sub(rotqt[0:64, col], t1a[:, :], t1b[:, :])
                t2a = p_tt.tile([64, 512], BF16, tag="c")
                t2b = p_tt.tile([64, 512], BF16, tag="d")
                nc.vector.tensor_mul(t2a[:, :], qr, s2n)
                nc.vector.tensor_mul(t2b[:, :], qi, c2n)
                nc.vector.tensor_add(rotqt[64:128, col], t2a[:, :], t2b[:, :])

        def attn_chunk(c):
            KC = (c + 1) * QPC
            q0 = c * CH
            for eta in range(NH):
                ps_o = p_psO.tile([128, CH], F32, tag="pso")
                ps_d = p_psD.tile([1, CH], F32, tag="psd")
                for kt in range(KC):
                    qo = max(0, (kt - c * QPC) * 128)
                    ps_s = p_psS.tile([128, CH], F32, tag="pss")
                    pt = p_pt.tile([128, CH], BF16, tag="pt")
                    nc.tensor.matmul(
                        ps_s[:, qo:CH],
                        rotqt[:, eta * T + kt * 128 : eta * T + kt * 128 + 128],
                        rotqt[:, eta * T + q0 + qo : eta * T + q0 + CH],
                        start=True,
                        stop=True,
                    )
                    if kt >= c * QPC:  # diagonal block: apply causal mask
                        nc.vector.tensor_add(
                            ps_s[:, qo : qo + 128],
                            ps_s[:, qo : qo + 128],
                            mkt_sb[:, :],
                        )
                    nc.scalar.activation(
                        pt[:, qo:CH],
                        ps_s[:, qo:CH],
                        mybir.ActivationFunctionType.Exp,
                        scale=scale,
                    )
                    nc.tensor.matmul(
                        ps_o[:, qo:CH],
                        v_sb[:, kt * DQ + eta * 128 : kt * DQ + eta * 128 + 128],
                        pt[:, qo:CH],
                        start=(kt == 0),
                        stop=(kt == KC - 1),
                    )
                    nc.tensor.matmul(
                        ps_d[:, qo:CH],
                        ones_sb[:, :],
                        pt[:, qo:CH],
                        start=(kt == 0),
                        stop=(kt == KC - 1),
                    )
                rcp = p_rcp.tile([1, CH], F32, tag="rcp")
                rcpb = p_rcp.tile([128, CH], F32, tag="rcpb")
                nc.vector.reciprocal_approx_fast(rcp[:, :], ps_d[:, :])
                nc.gpsimd.partition_broadcast(rcpb[:, :], rcp[:, :])
                otc = p_otc.tile([128, CH], BF16, tag="otc")
                nc.vector.tensor_mul(otc[:, :], ps_o[:, :], rcpb[:, :])
                nc.sync.dma_start(
                    out=ag_in[eta][c, :, :], in_=otc[:, :]
                )
                nc.gpsimd.collective_compute(
                    "AllGather",
                    mybir.AluOpType.bypass,
                    replica_groups=PAIRS,
                    ins=[ag_in[eta][c : c + 1, :, :].opt()],
                    outs=[ag_out[eta][c : c + 1, :, :, :].opt()],
                )

        for n in range(NQ):
            xt_sb = load_xt_quarter(n, nc.gpsimd if n % 2 else nc.sync)
            q_quarter(n, xt_sb)
            attn_chunk(n)

        # release all phase-1/2 pools (LIFO per space) before phase 3 opens
        for name in ("psD", "psO", "psS", "otc", "rcp", "pt", "ttmp",
                     "projps", "misc", "xt", "wqt", "vsb", "rotqt", "c2s2"):
            pclose(name)

        # ---------------- phase 3: gather + output projection -------------
        p_ofull = popen("ofull", bufs=1)
        p_wot = popen("wot", bufs=2)
        p_osb = popen("osb", bufs=3)
        p_psOut = popen("psOut", bufs=2, space="PSUM")

        # my-half tokens are chunks 2*h_idx and 2*h_idx+1 (dynamic indices)
        o_full = p_ofull.tile([128, 2 * NH * TH], BF16, tag="ofull")
        for r16 in range(2 * NH):
            eta = r16 % NH
            src_i = h_idx if r16 < NH else peer_i
            for j in range(2):
                ck = 2 * h_idx + j
                nc.gpsimd.dma_start(
                    out=o_full[:, r16 * TH + j * CH : r16 * TH + (j + 1) * CH],
                    in_=ag_out[eta][
                        bass_mod.ds(ck, 1), bass_mod.ds(src_i, 1), :, :
                    ],
                )

        NDO = D // 512
        for do in range(NDO):
            wot_sb = p_wot.tile([128, ND * 512], BF16, tag="wot")
            nc.sync.dma_start(
                out=wot_sb[:, :].rearrange("p (dk c) -> p dk c", dk=ND),
                in_=wot[:, do * 512 : (do + 1) * 512].rearrange(
                    "(dk p) c -> p dk c", dk=ND
                ),
            )
            for tb8 in range(TH // 128):
                ps_out = p_psOut.tile([128, 512], F32, tag="psout")
                for r16 in range(2 * NH):
                    nc.tensor.matmul(
                        ps_out[:, :],
                        o_full[:, r16 * TH + tb8 * 128 : r16 * TH + tb8 * 128 + 128],
                        wot_sb[:, r16 * 512 : (r16 + 1) * 512],
                        start=(r16 == 0),
                        stop=(r16 == 2 * NH - 1),
                    )
                osb = p_osb.tile([128, 512], F32, tag="osb")
                nc.vector.tensor_copy(osb[:, :], ps_out[:, :])
                nc.sync.dma_start(
                    out=out[tb8 * 128 : (tb8 + 1) * 128, do * 512 : (do + 1) * 512],
                    in_=osb[:, :],
                )

        for name in reversed(list(_cm)):
            pclose(name)

    nc.finalize()
    return nc


def host_prep(T, D, H, x, wq, wv, wo, mask, freqs_cos, freqs_sin):
    """Build per-core in_maps (host-side layout/dtype prep only)."""
    import ml_dtypes

    bf16 = ml_dtypes.bfloat16
    HD = 128
    NH = H // 2
    DQ = NH * HD
    mkt = np.ascontiguousarray(np.asarray(mask, np.float32).reshape(T, T)[:128, :128].T)
    fcn = np.asarray(freqs_cos, np.float32)  # [T, 64]
    fsn = np.asarray(freqs_sin, np.float32)
    fcs = np.ascontiguousarray(np.concatenate([fcn.T, fsn.T], axis=1))  # [64, 2T]
    # deinterleave permutation: within each head block, (r0,r1,..,i0,i1,..)
    perm = np.concatenate(
        [hb * 128 + np.r_[0:128:2, 1:128:2] for hb in range(NH)]
    )
    wot_full = np.asarray(wo, np.float32).T  # [din2, dout]
    in_maps = []
    for c in range(N_CORES):
        b, h = c // 2, c % 2
        rows = slice(h * DQ, (h + 1) * DQ)
        wqt_c = np.asarray(wq[rows], np.float32).T[:, perm]
        # o_full rows are in local head order (own heads first): permute wot
        wot_c = np.concatenate(
            [wot_full[h * DQ : (h + 1) * DQ], wot_full[(1 - h) * DQ : (2 - h) * DQ]],
            axis=0,
        )
        in_maps.append(
            {
                "xt": np.ascontiguousarray(np.asarray(x[b], np.float32).T.astype(bf16)),
                "wqt": np.ascontiguousarray(wqt_c.astype(bf16)),
                "wvt": np.ascontiguousarray(
                    np.asarray(wv[rows], np.float32).T.astype(bf16)
                ),
                "wot": np.ascontiguousarray(wot_c.astype(bf16)),
                "maskt": mkt,
                "fcs": fcs,
            }
        )
    return in_maps


_NC_CACHE = {}


def run(T, D, H, inputs, trace=False):
    from concourse.bass_utils import run_bass_kernel_spmd

    key = (T, D, H)
    if key not in _NC_CACHE:
        _NC_CACHE[key] = build_nc(T, D, H)
    nc = _NC_CACHE[key]
    in_maps = host_prep(
        T, D, H,
        inputs["x"], inputs["wq"], inputs["wv"], inputs["wo"],
        inputs["mask"], inputs["freqs_cos"], inputs["freqs_sin"],
    )
    res = run_bass_kernel_spmd(nc, in_maps, core_ids=list(range(N_CORES)), trace=trace)
    B_ = np.asarray(inputs["x"]).shape[0]
    TH = T // 2
    out = np.empty((B_, T, D), np.float32)
    for c in range(N_CORES):
        b, h = c // 2, c % 2
        out[b, h * TH : (h + 1) * TH, :] = res.results[c]["out"]
    return out, res


def kernel(**inputs):
    out, _ = run(T, D, H, inputs, trace=False)
    return out
